# revision 1
# baseline (speedup 1.0000x reference)
"""Distributed single-head attention for TRN2 (8 NeuronCores).

Reference computation (per batch b):
    q = x @ Wq; k = x @ Wk; v = x @ Wv          (x: [S, E])
    s = (q @ k.T) / sqrt(DK) - 1e15 * mask
    out = softmax(s, axis=-1) @ v               ([S, DV])

Sharding: 8 cores = 4 batches x 2 sequence halves. Each core computes
attention for 1024 queries of one batch; K/V are recomputed per core from
the full sequence (cheap vs. the attention matmuls, avoids collectives).

Per-core layout choices (host prepares these in kernel()):
  - xt  [E, S]  bf16: x_b^T with the sequence permuted so this core's
                query half occupies columns [0, 1024). K/V are computed
                over the permuted order, which is harmless because
                softmax-attention is permutation invariant over keys.
  - wq  [E, DK] bf16: Wq pre-scaled by 1/sqrt(DK).
  - mt  [S, SQ] bf16: mask[b, q0:q0+SQ, :] transposed to [key, query]
                with keys permuted identically to xt's columns.
  - out [P, SQ] f32: output queries grouped by 128: out[p, c*128+d] =
                attention_out[q0 + c*128 + p, d]; host re-interleaves.

On-core dataflow (all matmul contractions on the 128-partition dim):
  QT[d,q], KT[d,k], VT[d,k] projections -> DMA-transpose VT -> V[k,d]
  per key-tile t: ST[k128,q] = KT_t^T QT (PE) -> P = exp(ST) bf16 (ACT)
  -> zero masked entries (DVE copy_predicated) -> rowsum via ones-matmul
  and OT[dv,q] += V_t^T P (PE, PSUM accumulate). Epilogue transposes the
  rowsum into partition-major [128, 8] so the reciprocal runs on 8
  elements per lane (a [1, 1024] reciprocal costs 6.5us), then
  transposes OT chunks to [q, dv] and scales by 1/rowsum as a
  per-partition scalar.
"""

import math
from contextlib import ExitStack

import ml_dtypes
import numpy as np

import concourse.bass as bass
import concourse.bass_utils as _bass_utils
import concourse.tile as tile
from concourse import bacc, masks, mybir
from concourse.bass_utils import run_bass_kernel_spmd

# Note: tried forcing walrus --enable-ldw-opt=true to dedup the per-matmul
# LDWEIGHTS (~107ns each); walrus rejects it ("InstLdweights is not compatible
# with LDW optimization") because bacc emits standalone Ldweights when moving
# matmul waits. Keeping the default.
del _bass_utils

B, S, E, DK, DV = 4, 2048, 1024, 128, 128
SQ = S // 2  # queries per core
P = 128  # SBUF partitions
EC = E // P  # contraction chunks for projections
KTILES = S // P  # key tiles
QC = SQ // P  # query chunks (epilogue)

f32 = mybir.dt.float32
bf16 = mybir.dt.bfloat16

# test.py pokes these to get profiling info
TRACE = False
LAST_RESULT = None


def build():
    nc = bacc.Bacc()
    xt = nc.declare_dram_parameter("xt", [E, S], bf16, isOutput=False)
    # weights arrive host-packed as [p, c*DK+d] = W[c*128+p, d] so the load
    # is one fully-contiguous DMA (2KB/partition descriptors)
    wq = nc.declare_dram_parameter("wq", [P, EC * DK], bf16, isOutput=False)
    wk = nc.declare_dram_parameter("wk", [P, EC * DK], bf16, isOutput=False)
    wv = nc.declare_dram_parameter("wv", [P, EC * DV], bf16, isOutput=False)
    mt = nc.declare_dram_parameter("mt", [S, SQ], bf16, isOutput=False)
    out = nc.declare_dram_parameter("out", [P, SQ], f32, isOutput=True)

    with ExitStack() as ctx:
        tc = ctx.enter_context(tile.TileContext(nc))
        const_pool = ctx.enter_context(tc.tile_pool(name="const", bufs=1))
        in_pool = ctx.enter_context(tc.tile_pool(name="inputs", bufs=1))
        proj_sb = ctx.enter_context(tc.tile_pool(name="proj", bufs=1))
        p_pool = ctx.enter_context(tc.tile_pool(name="p", bufs=4))
        stat = ctx.enter_context(tc.tile_pool(name="stat", bufs=1))
        proj_ctx = ctx.enter_context(ExitStack())
        proj_ps = proj_ctx.enter_context(
            tc.tile_pool(name="proj_ps", bufs=8, space="PSUM")
        )

        ones_col = const_pool.tile([P, 1], bf16)
        nc.gpsimd.memset(ones_col[:], 1.0)
        zeros_p = const_pool.tile([P, SQ], bf16)
        nc.gpsimd.memset(zeros_p[:], 0.0)
        ident = const_pool.tile([P, P], f32)
        masks.make_identity(nc, ident[:])

        # --- weights + x chunks. One strided DMA per weight keeps the sync
        # sequencer's ~330ns/dma_start issue serialization off the critical
        # path; issue order wq, x0, wk, wv, x1.. matches first-use order.
        w_sb = {}
        for name, w in (("wq", wq), ("wk", wk), ("wv", wv)):
            wt = in_pool.tile([P, EC * DK], bf16, tag=name)
            w_sb[name] = wt

        x_sb = []
        for c in range(EC):
            xc = in_pool.tile([P, S], bf16, tag=f"x{c}")
            x_sb.append(xc)

        nc.sync.dma_start(w_sb["wq"][:], wq[:, :])
        nc.sync.dma_start(x_sb[0][:], xt[0:P, :])
        nc.sync.dma_start(w_sb["wk"][:], wk[:, :])
        nc.sync.dma_start(w_sb["wv"][:], wv[:, :])
        for c in range(1, EC):
            nc.sync.dma_start(x_sb[c][:], xt[c * P : (c + 1) * P, :])

        # --- projections: QT [d, q], KT [d, k], VT [d, k] (bf16 in SBUF) ---
        # Contraction chunk c is the outer loop so the first matmul only
        # needs x chunk 0, not all eight.
        qt_sb = proj_sb.tile([P, SQ], bf16)
        kt_sb = proj_sb.tile([P, S], bf16)
        vt_sb = proj_sb.tile([P, S], bf16)

        # QT and KT interleaved per x-chunk: ~1.9us of matmuls per chunk
        # keeps PE just behind the ~1.4us/chunk DMA arrival rate. VT follows
        # (x already resident by then).
        def alloc_ps(n):
            pss = []
            for j in range(n):
                ps = proj_ps.tile([P, 512], f32, tag="pps")
                pss.append(ps)
            return pss

        def proj_mm(pss, w_tile, c, nj):
            for j in range(nj):
                nc.tensor.matmul(
                    pss[j][:],
                    w_tile[:, c * DK : (c + 1) * DK],
                    x_sb[c][:, j * 512 : (j + 1) * 512],
                    start=(c == 0),
                    stop=(c == EC - 1),
                )

        def proj_copy(dst, pss):
            for j, ps in enumerate(pss):
                nc.vector.tensor_copy(dst[:, j * 512 : (j + 1) * 512], ps[:])

        qt_ps = alloc_ps(2)
        kt_ps = alloc_ps(4)
        for c in range(EC):
            proj_mm(qt_ps, w_sb["wq"], c, 2)
            proj_mm(kt_ps, w_sb["wk"], c, 4)
        proj_copy(qt_sb, qt_ps)
        proj_copy(kt_sb, kt_ps)
        vt_ps = alloc_ps(4)
        for c in range(EC):
            proj_mm(vt_ps, w_sb["wv"], c, 4)
        proj_copy(vt_sb, vt_ps)
        proj_ctx.close()  # free projection PSUM banks for the attention loop

        # --- V natural layout [k, dv] via DMA transpose on the sync stream
        # (sync has nothing left to issue afterward except output stores, so
        # blocking on vt_sb readiness is harmless) ---
        v_sb = proj_sb.tile([P, S], bf16)  # tile t at columns [t*DV, (t+1)*DV)
        for t in range(KTILES):
            nc.sync.dma_start_transpose(
                v_sb[:, t * DV : (t + 1) * DV], vt_sb[:, t * P : (t + 1) * P]
            )

        # --- mask loads on the GPSIMD SWDGE stream (third parallel issuer);
        # needed only from the attention loop onward ---
        m_sb = []
        for t in range(KTILES):
            mtile = in_pool.tile([P, SQ], bf16, tag=f"m{t}")
            m_sb.append(mtile)
            nc.gpsimd.dma_start(mtile[:], mt[t * P : (t + 1) * P, :])

        st_ps = ctx.enter_context(tc.tile_pool(name="st_ps", bufs=2, space="PSUM"))
        ot_ps = ctx.enter_context(tc.tile_pool(name="ot_ps", bufs=1, space="PSUM"))
        rs_pool = ctx.enter_context(tc.tile_pool(name="rs_ps", bufs=1, space="PSUM"))

        # --- attention over key tiles ---
        ot = ot_ps.tile([P, SQ], f32)  # OT [dv, q] accumulator
        rs = rs_pool.tile([1, SQ], f32)  # rowsum of masked exp(scores)
        for t in range(KTILES):
            st = st_ps.tile([P, SQ], f32, tag="st")  # [k128, q]
            for j in range(2):
                nc.tensor.matmul(
                    st[:, j * 512 : (j + 1) * 512],
                    kt_sb[:, t * P : (t + 1) * P],
                    qt_sb[:, j * 512 : (j + 1) * 512],
                    start=True,
                    stop=True,
                )
            p = p_pool.tile([P, SQ], bf16, tag="p")
            nc.scalar.activation(p[:], st[:], mybir.ActivationFunctionType.Exp)
            # zero the masked entries: exp(s - 1e15*m) == exp(s) * (1 - m)
            nc.vector.copy_predicated(
                p[:], m_sb[t][:].bitcast(mybir.dt.uint16), zeros_p[:]
            )
            for j in range(2):
                nc.tensor.matmul(
                    rs[:, j * 512 : (j + 1) * 512],
                    ones_col[:],
                    p[:, j * 512 : (j + 1) * 512],
                    start=(t == 0),
                    stop=(t == KTILES - 1),
                )
                nc.tensor.matmul(
                    ot[:, j * 512 : (j + 1) * 512],
                    v_sb[:, t * DV : (t + 1) * DV],
                    p[:, j * 512 : (j + 1) * 512],
                    start=(t == 0),
                    stop=(t == KTILES - 1),
                )

        # --- epilogue: normalize in [q, dv] layout ---
        # rowsum [1, SQ] -> SBUF -> PE-transpose to [128, QC] so reciprocal
        # runs on QC elements per lane instead of SQ on one lane.
        rs_sb = stat.tile([1, SQ], f32)
        nc.scalar.copy(rs_sb[:], rs[:])
        rsT = st_ps.tile([P, QC], f32, tag="st")
        for c in range(QC):
            nc.tensor.transpose(
                rsT[:, c : c + 1],
                rs_sb[0:1, c * P : (c + 1) * P],
                ident[0:1, 0:1],
            )
        rcpT = stat.tile([P, QC], f32)
        nc.vector.reciprocal(rcpT[:], rsT[:])

        ot_sb = stat.tile([P, SQ], f32)
        o_ps = st_ps.tile([P, SQ], f32, tag="st")
        o_sb = stat.tile([P, SQ], f32)
        # staged: copies, then transposes, then mults — interleaving PE
        # writes with DVE reads of the same PSUM bank forces serialization
        for c in range(QC):
            sl = slice(c * P, (c + 1) * P)
            nc.scalar.copy(ot_sb[:, sl], ot[:, sl])
        for c in range(QC):
            sl = slice(c * P, (c + 1) * P)
            nc.tensor.transpose(o_ps[:, sl], ot_sb[:, sl], ident[:])
        for c in range(QC):
            sl = slice(c * P, (c + 1) * P)
            nc.vector.tensor_scalar_mul(o_sb[:, sl], o_ps[:, sl], rcpT[:, c : c + 1])
            nc.sync.dma_start(out[:, sl], o_sb[:, sl])

    nc.compile()
    return nc


_NC_CACHE = None


def kernel(inputs, mask, Wq, Wk, Wv):
    global _NC_CACHE, LAST_RESULT
    inputs = np.asarray(inputs)
    mask = np.asarray(mask)
    bf = ml_dtypes.bfloat16
    scale = np.float32(1.0 / math.sqrt(DK))

    def pack_w(w):  # [E, DK] -> [p, c*DK+d] = w[c*128+p, d]
        w = np.asarray(w).astype(bf)
        return np.ascontiguousarray(
            w.reshape(EC, P, DK).transpose(1, 0, 2).reshape(P, EC * DK)
        )

    wq_h = pack_w(np.asarray(Wq) * scale)
    wk_h = pack_w(Wk)
    wv_h = pack_w(Wv)

    if _NC_CACHE is None:
        _NC_CACHE = build()
    nc = _NC_CACHE

    in_maps = []
    for core in range(8):
        b, h = divmod(core, 2)
        q0 = h * SQ
        idx = np.r_[q0:S, 0:q0]  # rotate so this core's queries come first
        xb = inputs[b]  # [S, E] f32
        xt_core = np.ascontiguousarray(xb[idx].T).astype(bf)  # [E, S]
        mt_core = np.ascontiguousarray(
            mask[b, q0 : q0 + SQ, :][:, idx].T
        ).astype(bf)  # [S, SQ]
        in_maps.append(
            {"xt": xt_core, "wq": wq_h, "wk": wk_h, "wv": wv_h, "mt": mt_core}
        )

    res = run_bass_kernel_spmd(nc, in_maps, list(range(8)), trace=TRACE)
    LAST_RESULT = res
    outp = np.empty((B, S, DV), np.float32)
    for core in range(8):
        b, h = divmod(core, 2)
        q0 = h * SQ
        o = np.asarray(res.results[core]["out"])  # [P, SQ]
        # out[p, c*128 + d] = attention_out[q0 + c*128 + p, d]
        outp[b, q0 : q0 + SQ, :] = (
            o.reshape(P, QC, DV).transpose(1, 0, 2).reshape(SQ, DV)
        )
    return outp



# revision 6
# speedup vs baseline: 1.0409x; 1.0409x over previous
"""Distributed single-head attention for TRN2 (8 NeuronCores).

Reference computation (per batch b):
    q = x @ Wq; k = x @ Wk; v = x @ Wv          (x: [S, E])
    s = (q @ k.T) / sqrt(DK) - 1e15 * mask
    out = softmax(s, axis=-1) @ v               ([S, DV])

Sharding: 8 cores = 4 batches x 2 sequence halves. Each core computes
attention for 1024 queries of one batch; K/V are recomputed per core from
the full sequence.

Host-prepared per-core layout:
  - xt  [E, S]  bf16: x_b^T, sequence permuted so this core's queries come
                first. K/V over the permuted order (softmax is permutation
                invariant over keys).
  - wq/wk/wv [P, EC*DK] bf16: weights packed [p, c*DK+d] = W[c*128+p, d]
                (wq pre-scaled by 1/sqrt(DK)).
  - mi  [S, SQ] bf16: (1 - mask)[b, q0:q0+SQ, :].T (keys permuted like xt).
  - out [P, SQ] bf16: out[p, c*128+d] = attention_out[q0+c*128+p, d].

Schedule (all engine queues are FIFO; ordering below is load-bearing):
  - sync HWDGE queue: wq, x0, wk, wv, x1..x7 loads, then 16 V-tile
    dma transposes, then the single output store. Masks go on the gpsimd
    SWDGE queue but only after a dummy dep on x7 — otherwise they steal
    half the HBM bandwidth and starve the projections (baseline's bug).
  - projections: chunk loop c=0..7 computes VT slabs 0-1 + QT + all 4 KT
    slabs (8 matmuls, exactly 8 PSUM banks), then VT slabs 2-3 dense.
    V transposes start as soon as vt slab copies land.
  - attention loop, software-pipelined with lead 2: PE issue order is
    score(t+2) BEFORE rowsum/ot(t), so the PE queue always has independent
    work while the exp(t) -> mask-mult(t) chain completes. Masking is a
    plain tensor_mul with the host-inverted mask (2x DVE mode) instead of
    copy_predicated (1x mode).
  - epilogue: per-128-query-chunk pipeline copy(ACT) -> transpose(PE) ->
    scale(DVE) with per-chunk PSUM tiles, one contiguous bf16 output DMA.
"""

import math
from contextlib import ExitStack

import ml_dtypes
import numpy as np

import concourse.bass as bass
import concourse.tile as tile
from concourse import bacc, masks, mybir
from concourse.bass_utils import run_bass_kernel_spmd

B, S, E, DK, DV = 4, 2048, 1024, 128, 128
SQ = S // 2  # queries per core
P = 128  # SBUF partitions
EC = E // P  # contraction chunks for projections
KTILES = S // P  # key tiles
QC = SQ // P  # query chunks (epilogue)
LEAD = 2  # score-matmul software-pipeline depth

f32 = mybir.dt.float32
bf16 = mybir.dt.bfloat16

# test.py pokes these to get profiling info
TRACE = False
LAST_RESULT = None


def build():
    nc = bacc.Bacc()
    xt = nc.declare_dram_parameter("xt", [E, S], bf16, isOutput=False)
    wq = nc.declare_dram_parameter("wq", [P, EC * DK], bf16, isOutput=False)
    wk = nc.declare_dram_parameter("wk", [P, EC * DK], bf16, isOutput=False)
    wv = nc.declare_dram_parameter("wv", [P, EC * DV], bf16, isOutput=False)
    mi = nc.declare_dram_parameter("mi", [S, SQ], bf16, isOutput=False)
    out = nc.declare_dram_parameter("out", [P, SQ], bf16, isOutput=True)

    with ExitStack() as ctx:
        tc = ctx.enter_context(tile.TileContext(nc))
        const_pool = ctx.enter_context(tc.tile_pool(name="const", bufs=1))
        in_pool = ctx.enter_context(tc.tile_pool(name="inputs", bufs=1))
        proj_sb = ctx.enter_context(tc.tile_pool(name="proj", bufs=1))
        p_pool = ctx.enter_context(tc.tile_pool(name="p", bufs=3))
        stat = ctx.enter_context(tc.tile_pool(name="stat", bufs=1))
        proj_ctx = ctx.enter_context(ExitStack())
        proj_ps = proj_ctx.enter_context(
            tc.tile_pool(name="proj_ps", bufs=8, space="PSUM")
        )

        ones_col = const_pool.tile([P, 1], bf16)
        nc.gpsimd.memset(ones_col[:], 1.0)
        ident = const_pool.tile([P, P], f32)
        masks.make_identity(nc, ident[:])
        ident_bf = const_pool.tile([P, P], bf16)
        nc.vector.tensor_copy(ident_bf[:], ident[:])
        # preload the exp table set off the critical path
        warm = const_pool.tile([1, 2], f32)
        nc.gpsimd.memset(warm[:], 0.0)
        nc.scalar.activation(warm[:], warm[:], mybir.ActivationFunctionType.Exp)

        # --- input loads: weights + x chunks on the sync HWDGE queue, in
        # first-use order. Masks are SWDGE (gpsimd) but gated behind x7.
        w_sb = {}
        for name in ("wq", "wk", "wv"):
            w_sb[name] = in_pool.tile([P, EC * DK], bf16, tag=name, name=f"w_{name}")
        x_sb = []
        for c in range(EC):
            x_sb.append(in_pool.tile([P, S], bf16, tag=f"x{c}", name=f"x{c}"))

        nc.sync.dma_start(w_sb["wq"][:], wq[:, :])
        nc.sync.dma_start(x_sb[0][:], xt[0:P, :])
        nc.sync.dma_start(w_sb["wk"][:], wk[:, :])
        nc.sync.dma_start(w_sb["wv"][:], wv[:, :])
        for c in range(1, EC):
            nc.sync.dma_start(x_sb[c][:], xt[c * P : (c + 1) * P, :])

        # masks: delayed behind x7 via a dummy gpsimd read of x_sb[7]
        delay_probe = const_pool.tile([1, 2], bf16)
        nc.gpsimd.tensor_copy(delay_probe[:], x_sb[EC - 1][0:1, 0:2])
        m_sb = []
        for t in range(KTILES):
            mtile = in_pool.tile([P, SQ], bf16, tag=f"m{t}")
            m_sb.append(mtile)
            nc.gpsimd.dma_start(mtile[:], mi[t * P : (t + 1) * P, :])

        # --- projections ---
        # SBUF targets: qt [d, q]; kt/vt as 4 slabs of [d, 512] each;
        # v as 16 tiles [k, dv] (via DMA transpose of vt).
        qt_sb = proj_sb.tile([P, SQ], bf16)
        kt_sb = [proj_sb.tile([P, 512], bf16, tag=f"kt{s}", name=f"kt{s}") for s in range(4)]
        vt_sb = [proj_sb.tile([P, 512], bf16, tag=f"vt{s}", name=f"vt{s}") for s in range(4)]
        v_sb = [proj_sb.tile([P, DV], bf16, tag=f"v{t}", name=f"v{t}") for t in range(KTILES)]

        # PSUM: one pool, 8 banks, tiles rotate in allocation order.
        # Phase A (chunk loop): vtA0 vtA1 qt0 qt1 kt0..kt3 = 8 banks.
        # Phase B: vtB0 vtB1 reuse vtA0/vtA1 banks (WAR on their copies).
        ps = {}
        for tag in ("vtA0", "vtA1", "qt0", "qt1", "kt0", "kt1", "kt2", "kt3"):
            ps[tag] = proj_ps.tile([P, 512], f32, tag="pps", name=f"ps_{tag}")

        def wslice(w, c):
            return w[:, c * DK : (c + 1) * DK]

        for c in range(EC):
            st_flags = dict(start=(c == 0), stop=(c == EC - 1))
            for j in range(2):  # VT slabs 0-1
                nc.tensor.matmul(
                    ps[f"vtA{j}"][:],
                    wslice(w_sb["wv"], c),
                    x_sb[c][:, j * 512 : (j + 1) * 512],
                    **st_flags,
                )
            for j in range(2):  # QT (queries are columns [0, 1024))
                nc.tensor.matmul(
                    ps[f"qt{j}"][:],
                    wslice(w_sb["wq"], c),
                    x_sb[c][:, j * 512 : (j + 1) * 512],
                    **st_flags,
                )
            for j in range(4):  # KT all 4 slabs
                nc.tensor.matmul(
                    ps[f"kt{j}"][:],
                    wslice(w_sb["wk"], c),
                    x_sb[c][:, j * 512 : (j + 1) * 512],
                    **st_flags,
                )

        # PSUM -> SBUF copies, split across DVE and ACT so neither is the
        # critical path: DVE gets vt slabs (feeds the V transposes), ACT
        # gets qt (feeds the first score matmuls) then kt.
        nc.vector.tensor_copy(vt_sb[0][:], ps["vtA0"][:])
        nc.scalar.copy(qt_sb[:, 0:512], ps["qt0"][:])
        nc.vector.tensor_copy(vt_sb[1][:], ps["vtA1"][:])
        nc.scalar.copy(qt_sb[:, 512:1024], ps["qt1"][:])
        nc.vector.tensor_copy(kt_sb[1][:], ps["kt1"][:])
        nc.scalar.copy(kt_sb[0][:], ps["kt0"][:])
        nc.vector.tensor_copy(kt_sb[3][:], ps["kt3"][:])
        nc.scalar.copy(kt_sb[2][:], ps["kt2"][:])

        # VT slabs 2-3 (phase B) on the banks freed by the vtA copies
        ps["vtB0"] = proj_ps.tile([P, 512], f32, tag="pps", name="ps_vtB0")
        ps["vtB1"] = proj_ps.tile([P, 512], f32, tag="pps", name="ps_vtB1")
        for c in range(EC):
            st_flags = dict(start=(c == 0), stop=(c == EC - 1))
            for j in range(2):
                nc.tensor.matmul(
                    ps[f"vtB{j}"][:],
                    wslice(w_sb["wv"], c),
                    x_sb[c][:, (2 + j) * 512 : (3 + j) * 512],
                    **st_flags,
                )
        nc.vector.tensor_copy(vt_sb[2][:], ps["vtB0"][:])
        nc.vector.tensor_copy(vt_sb[3][:], ps["vtB1"][:])

        # V natural layout [k, dv] via xbar DMA transposes (sync queue,
        # issued after all input loads; they execute as vt slabs land)
        for t in range(KTILES):
            s, o = divmod(t, 4)
            nc.sync.dma_start_transpose(
                v_sb[t][:], vt_sb[s][:, o * P : (o + 1) * P]
            )

        proj_ctx.close()  # free projection PSUM banks for the loop

        ot_ps = ctx.enter_context(tc.tile_pool(name="ot_ps", bufs=1, space="PSUM"))
        rs_pool = ctx.enter_context(tc.tile_pool(name="rs_ps", bufs=1, space="PSUM"))
        st_ctx = ctx.enter_context(ExitStack())
        st_ps = st_ctx.enter_context(tc.tile_pool(name="st_ps", bufs=2, space="PSUM"))

        ot = ot_ps.tile([P, SQ], f32)  # OT [dv, q] accumulator
        rs = rs_pool.tile([1, SQ], f32)  # rowsum of masked exp(scores)

        st_tiles = []

        def score_mms(t):
            st = st_ps.tile([P, SQ], f32, tag="st")
            st_tiles.append(st)
            s, o = divmod(t, 4)
            for j in range(2):
                nc.tensor.matmul(
                    st[:, j * 512 : (j + 1) * 512],
                    kt_sb[s][:, o * P : (o + 1) * P],
                    qt_sb[:, j * 512 : (j + 1) * 512],
                    start=True,
                    stop=True,
                )

        # prologue: scores for tiles 0..LEAD-1
        for t in range(LEAD):
            score_mms(t)

        for t in range(KTILES):
            st = st_tiles[t]
            p = p_pool.tile([P, SQ], bf16, tag="p")
            nc.scalar.activation(p[:], st[:], mybir.ActivationFunctionType.Exp)
            # zero masked entries: p *= (1 - mask)
            nc.vector.tensor_mul(p[:], p[:], m_sb[t][:])
            if t + LEAD < KTILES:
                score_mms(t + LEAD)
            for j in range(2):
                nc.tensor.matmul(
                    rs[:, j * 512 : (j + 1) * 512],
                    ones_col[:],
                    p[:, j * 512 : (j + 1) * 512],
                    start=(t == 0),
                    stop=(t == KTILES - 1),
                )
            for j in range(2):
                nc.tensor.matmul(
                    ot[:, j * 512 : (j + 1) * 512],
                    v_sb[t][:],
                    p[:, j * 512 : (j + 1) * 512],
                    start=(t == 0),
                    stop=(t == KTILES - 1),
                )

        # --- epilogue: normalize in [q, dv] layout ---
        # rowsum [1, SQ] -> SBUF -> PE-transpose chunks to [128, QC] ->
        # reciprocal; ot chunks: copy (ACT, ->bf16), PE transpose, DVE
        # scale by 1/rowsum; single contiguous bf16 output DMA.
        st_ctx.close()  # free the score PSUM banks for the epilogue
        epi_ps = ctx.enter_context(tc.tile_pool(name="epi_ps", bufs=2, space="PSUM"))
        rsT_pool = ctx.enter_context(tc.tile_pool(name="rsT_ps", bufs=1, space="PSUM"))

        rs_sb = stat.tile([1, SQ], f32)
        nc.scalar.copy(rs_sb[:], rs[:])
        rsT = rsT_pool.tile([P, QC], f32)
        for c in range(QC):
            nc.tensor.transpose(
                rsT[:, c : c + 1],
                rs_sb[0:1, c * P : (c + 1) * P],
                ident[0:1, 0:1],
            )
        rcpT = stat.tile([P, QC], f32)
        nc.vector.reciprocal(rcpT[:], rsT[:])

        ot_sb = [stat.tile([P, P], bf16, tag=f"ot{c}", name=f"ot_sb{c}") for c in range(QC)]
        o_sb = stat.tile([P, SQ], bf16)
        for c in range(QC):
            sl = slice(c * P, (c + 1) * P)
            nc.scalar.copy(ot_sb[c][:], ot[:, sl])
            o_ps = epi_ps.tile([P, P], bf16, tag="ops")
            nc.tensor.transpose(o_ps[:], ot_sb[c][:], ident_bf[:])
            nc.vector.tensor_scalar_mul(o_sb[:, sl], o_ps[:], rcpT[:, c : c + 1])
        nc.sync.dma_start(out[:, :], o_sb[:])

    nc.compile()
    return nc


_NC_CACHE = None


def kernel(inputs, mask, Wq, Wk, Wv):
    global _NC_CACHE, LAST_RESULT
    inputs = np.asarray(inputs)
    mask = np.asarray(mask)
    bf = ml_dtypes.bfloat16
    scale = np.float32(1.0 / math.sqrt(DK))

    def pack_w(w):  # [E, DK] -> [p, c*DK+d] = w[c*128+p, d]
        w = np.asarray(w).astype(bf)
        return np.ascontiguousarray(
            w.reshape(EC, P, DK).transpose(1, 0, 2).reshape(P, EC * DK)
        )

    wq_h = pack_w(np.asarray(Wq) * scale)
    wk_h = pack_w(Wk)
    wv_h = pack_w(Wv)

    if _NC_CACHE is None:
        _NC_CACHE = build()
    nc = _NC_CACHE

    minv = (1 - mask).astype(bf)  # [B, S, S], entries in {0, 1}
    in_maps = []
    for core in range(8):
        b, h = divmod(core, 2)
        q0 = h * SQ
        idx = np.r_[q0:S, 0:q0]  # rotate so this core's queries come first
        xb = inputs[b]  # [S, E] f32
        xt_core = np.ascontiguousarray(xb[idx].T).astype(bf)  # [E, S]
        mi_core = np.ascontiguousarray(
            minv[b, q0 : q0 + SQ, :][:, idx].T
        )  # [S, SQ]
        in_maps.append(
            {"xt": xt_core, "wq": wq_h, "wk": wk_h, "wv": wv_h, "mi": mi_core}
        )

    res = run_bass_kernel_spmd(nc, in_maps, list(range(8)), trace=TRACE)
    LAST_RESULT = res
    outp = np.empty((B, S, DV), np.float32)
    for core in range(8):
        b, h = divmod(core, 2)
        q0 = h * SQ
        o = np.asarray(res.results[core]["out"]).astype(np.float32)  # [P, SQ]
        # out[p, c*128 + d] = attention_out[q0 + c*128 + p, d]
        outp[b, q0 : q0 + SQ, :] = (
            o.reshape(P, QC, DV).transpose(1, 0, 2).reshape(SQ, DV)
        )
    return outp
